# revision 26
# baseline (speedup 1.0000x reference)
"""MultiHeadAttention forward on 8 TRN2 NeuronCores (batch*head parallel).

Problem: S=2048, B=2, E=1024, H=16 heads, D=64. Each core handles one batch
(b = core//4) and 4 consecutive heads ((core%4)*4 ...), as 2 head-pairs.

v3: float16 data path (numpy-sim metric 4.0e-3 vs the 2e-2 gate; fp16 matmuls
run 1 cycle/row with overlappable fast weight loads, halving DMA bytes and
killing the f32r weight-load serialization). The kernel is paced by the
Scalar engine's exp (~1.12us per [128,1024] strip, 128 strips = 143us floor);
everything else is arranged to keep that engine saturated:

  Warmup: ~7us of dummy matmuls raise the PE HAM clock gate (1.2 -> 2.4 GHz)
    while input DMAs stream; a dummy exp preloads the ACT spline table.
  DMA: weights arrive pre-shuffled so every transfer is fat-descriptor
    contiguous. wqk + x stream on the Activation HWDGE queue (free until the
    first exp), wv/wout/bias on the Sync queue in parallel.
  Phase A (upfront): only what phase B's first quarter needs: K chunks of
    pair 0 (fc2), Q chunk sb0 (fc0), and V. The other 11 projection groups
    run inside phase B's PE slack via the aux PSUM ring, two per quarter.
  Phase B per (pair, tq): per 128-key chunk i: 2 row-packed score matmuls
    [64-contraction, heads at rows 0-63/64-127], one exp over the combined
    [128,1024] PSUM strip (scale 1/8 folded in), 2 P@V accumulations with
    [V|1] stationary (the ones column yields softmax denominators in row 64).
    Score matmuls for chunk i+1 are emitted BEFORE P@V of chunk i so the PE
    queue never head-of-line blocks the next exp strip.
  Finalize: P@V + denominators staged to SBUF right after the last chunk
    (frees PSUM); the rest (denominator shuffle-DMA onto 128 partitions, DVE
    reciprocal, DRAM-bounce broadcast over 64 partitions, 2 multiplies) is
    emitted early in the NEXT quarter, as is out_proj (8 matmuls + copies +
    one batched 2-kick DMA out). K bias is softmax-invariant (dropped); V
    bias is folded into the output bias host-side; Q bias applied on-chip.
  Host: sums the 2x4 fp16 partials per batch in fp32 and adds the folded
    output bias.
"""
import os
import sys

if "/opt/trn_rl_repo" not in sys.path:
    sys.path.insert(0, "/opt/trn_rl_repo")

import numpy as np

import concourse.bass as bass
import concourse.tile as tile
from concourse import mybir
from concourse.bass_utils import run_bass_kernel_spmd

_LDWOPT = os.environ.get("LDWOPT", "0") == "1"
if _LDWOPT:
    import concourse.bass_utils as _bu

    _orig_run_command = _bu.run_command

    def _run_command_ldwopt(argv, **kw):
        argv = ["--enable-ldw-opt=true" if a == "--enable-ldw-opt=false" else a
                for a in argv]
        return _orig_run_command(argv, **kw)

    _bu.run_command = _run_command_ldwopt

S = 2048
B = 2
E = 1024
H = 16
D = 64
N_CORES = 8
F32 = mybir.dt.float32
F16 = mybir.dt.float16
EXP = mybir.ActivationFunctionType.Exp
COPY = mybir.ActivationFunctionType.Copy
SCALING = float(D) ** -0.5

NSCH = S // 128   # 16 s-chunks
NSB = S // 512    # 4 s-blocks
NEC = E // 128    # 8 e-chunks


def _split_excess_waits(nc, limit=1):
    """This walrus build accepts at most 2 sync-wait commands per instruction;
    hoist excess waits onto preceding same-engine NOPs (queue order preserves
    semantics)."""
    ctr = 0
    for f in nc.m.functions:
        for blk in f.blocks:
            insts = blk.instructions
            if not any(
                i.sync_info and i.sync_info.on_wait and len(i.sync_info.on_wait) > limit
                for i in insts
            ):
                continue
            out = []
            for inst in insts:
                si = inst.sync_info
                if si and si.on_wait and len(si.on_wait) > limit:
                    waits = list(si.on_wait)
                    excess, keep = waits[:-limit], waits[-limit:]
                    for i in range(0, len(excess), limit):
                        ctr += 1
                        nop = mybir.InstNoOp(name=f"waitsplit-nop-{ctr}")
                        nop.engine = inst.engine
                        nop.sync_info = mybir.SyncInfo(
                            on_wait=excess[i : i + limit], on_update=[]
                        )
                        nc.register_instruction(nop, overwrite=True)
                        out.append(nop)
                    si.on_wait = keep
                out.append(inst)
            blk.instructions.clear()
            blk.instructions.extend(out)
    return nc


def _build_nc():
    nc = bass.Bass()
    xT = nc.dram_tensor("xT", [128, NEC, S], F16, kind="ExternalInput")
    wqkAS = nc.dram_tensor("wqkAS", [128, NEC * 256], F16, kind="ExternalInput")
    wqkBS = nc.dram_tensor("wqkBS", [128, NEC * 256], F16, kind="ExternalInput")
    wvS = nc.dram_tensor("wvS", [128, NEC * 256], F16, kind="ExternalInput")
    woutS = nc.dram_tensor("woutS", [128, 2 * E], F16, kind="ExternalInput")
    bias_q = nc.dram_tensor("bias_q", [128, 2], F32, kind="ExternalInput")
    outT = nc.dram_tensor("outT", [2, E, S], F16, kind="ExternalOutput")
    # DRAM scratch for the reciprocal rows (SBUF->SBUF broadcast DMA is not
    # supported; DRAM->SBUF broadcast is): one row per (pair, tq).
    recd = nc.dram_tensor("recd", [8, 2, 512], F32, kind="Internal")

    with tile.TileContext(nc) as tc, \
         tc.tile_pool(name="xpool", bufs=1) as xpool, \
         tc.tile_pool(name="wpool", bufs=1) as wpool:
        # ---- input DMA kicks. The hardware DMA rings share ~0.38 MB/us of
        # aggregate bandwidth and the Activation ring cannot start before
        # ~11.5us, so the critical stream (wqk then x, in the exact ec order
        # the in-order QK accumulation consumes) rides the Sync queue; the
        # small late-use weights ride the Activation queue.
        xt = xpool.tile([128, NEC, S], F16)
        wqkA = wpool.tile([128, NEC, 256], F16)   # fc0 | fc2
        wqkB = wpool.tile([128, NEC, 256], F16)   # fc1 | fc3
        wv = wpool.tile([128, NEC, 256], F16)
        bq = wpool.tile([128, 2], F32)
        wout = wpool.tile([128, 2, E], F16)
        nc.sync.dma_start(
            out=wqkA, in_=wqkAS.rearrange("p (c f) -> p c f", c=NEC))
        for ec in range(NEC):
            nc.sync.dma_start(out=xt[:, ec, :], in_=xT[:, ec, :])
        nc.sync.dma_start(
            out=wqkB, in_=wqkBS.rearrange("p (c f) -> p c f", c=NEC))
        nc.scalar.dma_start(out=bq, in_=bias_q[:, :])
        nc.scalar.dma_start(
            out=wv, in_=wvS.rearrange("p (c f) -> p c f", c=NEC))
        nc.scalar.dma_start(
            out=wout, in_=woutS.rearrange("p (c f) -> p c f", c=2))

        def wsel(fc, ec):
            t = wqkA if fc in (0, 2) else wqkB
            return t[:, ec, bass.ds((fc // 2) * 128, 128)]

        # ---- warmup: dummy matmuls (~7us at the cold clock) raise the HAM
        # clock gate to 2.4 GHz while the input DMAs stream; a dummy exp
        # preloads the ACT spline table. No DMA dependencies.
        with tc.tile_pool(name="wupool", bufs=1) as wup, \
             tc.tile_pool(name="wupsum", bufs=2, space="PSUM") as wups:
            wu = wup.tile([128, 512], F16)
            with nc.named_scope(f"warmup_ldwopt{int(_LDWOPT)}"):
                nc.vector.memset(wu, 1.0)
            wusink = wup.tile([128, 512], F32)
            for w in range(16):
                ps = wups.tile([128, 512], F32, tag="wu")
                nc.tensor.matmul(ps, wu[:, 0:128], wu, start=True, stop=True)
                if w == 15:
                    nc.scalar.activation(wusink, ps, EXP, scale=0.001)

        with tc.tile_pool(name="qkpool", bufs=1) as qkpool, \
             tc.tile_pool(name="vapool", bufs=1) as vapool, \
             tc.tile_pool(name="attnpool", bufs=1) as attnpool, \
             tc.tile_pool(name="ppool", bufs=4) as ppool:
            ones64 = wpool.tile([128, 64], F32)
            nc.vector.memset(ones64, 1.0)
            onesr = wpool.tile([1, 64], F16)
            nc.vector.memset(onesr, 1.0)

            # persistent activations
            qk = qkpool.tile([128, 4, S], F16)         # Q^T (chunks 0-1), K^T (2-3)
            va = vapool.tile([128, NSCH, 4, 65], F16)  # V natural + ones col
            attn = attnpool.tile([128, 2, S], F16)     # attn^T normalized

            nc.vector.tensor_copy(
                va[:, :, :, 64:65],
                ones64.rearrange("p (c h) -> p c h", h=4).unsqueeze(3))

            qkg_state = {}

            def emit_qk_group(fc, sb, pool, tag, half=None):
                """QKV projection group; half=0/1 splits the 8-MM
                accumulation into two 4-MM batches (same psum tile) so a
                group fits a strip's PE slack without bursting."""
                if half in (None, 0):
                    qkg_state[(fc, sb)] = pool.tile(
                        [128, 512], F32, tag=tag, name=f"qk{fc}_{sb}")
                ps = qkg_state[(fc, sb)]
                ecs = range(NEC) if half is None else (
                    range(4) if half == 0 else range(4, NEC))
                for ec in ecs:
                    nc.tensor.matmul(
                        ps,
                        wsel(fc, ec),
                        xt[:, ec, bass.ts(sb, 512)],
                        start=(ec == 0), stop=(ec == NEC - 1))
                if half == 0:
                    return
                if fc < 2:  # Q chunk: add bias
                    nc.vector.tensor_scalar(
                        out=qk[:, fc, bass.ts(sb, 512)], in0=ps,
                        scalar1=bq[:, fc:fc + 1], scalar2=None,
                        op0=mybir.AluOpType.add)
                else:       # K chunk: bias dropped (softmax-invariant)
                    nc.vector.tensor_copy(qk[:, fc, bass.ts(sb, 512)], ps)

            def emit_v_group(i, pool, tag):
                ps = pool.tile([128, 512], F32, tag=tag, name=f"v{i}")
                for ec in range(NEC):
                    nc.tensor.matmul(
                        ps[:, 0:256],
                        xt[:, ec, bass.ts(i, 128)],
                        wv[:, ec, :],
                        start=(ec == 0), stop=(ec == NEC - 1))
                nc.vector.tensor_copy(
                    va[:, i, :, 0:64],
                    ps[:, 0:256].rearrange("p (h d) -> p h d", h=4))

            # ---- phase A (upfront): only what phase B quarter 0's first
            # strip needs; V and the other 11 projection groups are spread
            # through phase B's strip slots via the aux ring.
            # One PSUM slot per group and ec-wave emission order: each
            # arriving x chunk feeds every group immediately (the in-order
            # PE queue would otherwise serialize whole groups behind the
            # slowest chunk).
            with nc.named_scope("phaseA"), \
                 tc.tile_pool(name="apsum", bufs=1, space="PSUM") as apsum:
                agroups = [(0, 0), (2, 0), (2, 1)]
                aps = {
                    (fc, sb): apsum.tile([128, 512], F32, tag=f"a{fc}_{sb}",
                                         name=f"aps{fc}_{sb}")
                    for fc, sb in agroups
                }
                for ec in range(NEC):
                    for fc, sb in agroups:
                        nc.tensor.matmul(
                            aps[(fc, sb)],
                            wsel(fc, ec),
                            xt[:, ec, bass.ts(sb, 512)],
                            start=(ec == 0), stop=(ec == NEC - 1))
                for fc, sb in agroups:
                    if fc < 2:
                        nc.vector.tensor_scalar(
                            out=qk[:, fc, bass.ts(sb, 512)],
                            in0=aps[(fc, sb)],
                            scalar1=bq[:, fc:fc + 1], scalar2=None,
                            op0=mybir.AluOpType.add)
                    else:
                        nc.vector.tensor_copy(
                            qk[:, fc, bass.ts(sb, 512)], aps[(fc, sb)])

            # late work items: ("v", i) or ("qk", (fc, sb), half).
            # The in-order PE queue guarantees each lands before its
            # consumer; the strip placement only controls stalls.
            def qk_halves(fc, sb, s0, s1):
                return [(s0, ("qk", (fc, sb), 0)), (s1, ("qk", (fc, sb), 1))]

            late_slots = {
                0: [(i, ("v", i)) for i in range(NSCH)]
                   + qk_halves(2, 2, 2, 3) + qk_halves(2, 3, 5, 6)
                   + qk_halves(0, 1, 12, 13),
                1: qk_halves(0, 2, 7, 8) + qk_halves(0, 3, 11, 12),
                2: qk_halves(3, 0, 7, 8) + qk_halves(3, 1, 11, 12),
                3: qk_halves(3, 2, 6, 7) + qk_halves(3, 3, 8, 9)
                   + qk_halves(1, 0, 12, 13),
                4: qk_halves(1, 1, 7, 8) + qk_halves(1, 2, 11, 12),
                5: qk_halves(1, 3, 7, 8),
            }

            # ---- phase B + deferred finalize/out_proj ----
            with tc.tile_pool(name="unpool", bufs=3) as unpool, \
                 tc.tile_pool(name="fpool", bufs=2) as fpool, \
                 tc.tile_pool(name="opool", bufs=2) as opool, \
                 tc.tile_pool(name="scpsum", bufs=2, space="PSUM") as scp, \
                 tc.tile_pool(name="pvpsum", bufs=1, space="PSUM") as pvp, \
                 tc.tile_pool(name="auxpsum", bufs=2, space="PSUM") as auxp:

                def emit_fin_rest(pair, tq, un):
                    """Denominator -> reciprocal -> broadcast -> normalize."""
                    with nc.named_scope(f"fin{pair}_{tq}"):
                        recin = fpool.tile([128, 8], F32, tag="recin")
                        nc.sync.dma_start(out=recin, in_=un[64:65, :, :])
                        recw = fpool.tile([128, 8], F32, tag="recw")
                        nc.vector.reciprocal(recw, recin)
                        ridx = pair * 4 + tq
                        nc.sync.dma_start(out=recd[ridx], in_=recw)
                        recb = fpool.tile([64, 2, 512], F32, tag="recb")
                        nc.sync.dma_start(
                            out=recb,
                            in_=recd[ridx].unsqueeze(0)
                            .to_broadcast([64, 2, 512]))
                        for h in range(2):
                            prt = h * 64
                            nc.vector.tensor_mul(
                                attn[prt:prt + 64, pair,
                                     bass.ds(tq * 512, 512)],
                                un[0:64, h, :],
                                recb[:, h, :])

                oproj_state = {}

                def emit_oproj_chunk(pair, tq, fcs, tail=False):
                    toff = tq * 512
                    with nc.named_scope(f"oproj{pair}_{tq}"):
                        if fcs[0] == 0:
                            oproj_state["ocp"] = opool.tile(
                                [128, NEC, 512], F16, tag="ocp", name="ocp")
                        ocp = oproj_state["ocp"]
                        for fc in fcs:
                            ps = auxp.tile([128, 512], F32, tag="aux",
                                           name=f"op{pair}_{tq}_{fc}")
                            nc.tensor.matmul(
                                ps,
                                wout[:, pair, bass.ts(fc, 128)],
                                attn[:, pair, bass.ds(toff, 512)],
                                start=True, stop=True)
                            if tail and fc % 2 == 1:
                                nc.scalar.activation(ocp[:, fc, :], ps, COPY)
                            else:
                                nc.vector.tensor_copy(ocp[:, fc, :], ps)
                        if fcs[-1] == NEC - 1:
                            outv = outT[pair].rearrange(
                                "(c p) s -> p c s", p=128)
                            nk = 4 if tail else 2
                            step = NEC // nk
                            for k in range(nk):
                                nc.sync.dma_start(
                                    out=outv[:, bass.ds(k * step, step),
                                             bass.ds(toff, 512)],
                                    in_=ocp[:, bass.ds(k * step, step), :])

                def close_a(prev):
                    """Deferred P@V of chunk 14 (emitted after the next
                    quarter's first score pair)."""
                    pair, tq, pvA, pvB, hA, hB, p14, p15 = prev
                    nc.tensor.matmul(
                        pvA, va[:, NSCH - 2, hA, :], p14[:, 0:512],
                        start=False, stop=False)
                    nc.tensor.matmul(
                        pvB, va[:, NSCH - 2, hB, :], p14[:, 512:1024],
                        start=False, stop=False)

                def close_b(prev, tail=False):
                    """Deferred P@V of chunk 15 + SBUF staging; frees the
                    P@V PSUM banks for the next quarter's accumulation."""
                    pair, tq, pvA, pvB, hA, hB, p14, p15 = prev
                    nc.tensor.matmul(
                        pvA, va[:, NSCH - 1, hA, :], p15[:, 0:512],
                        start=False, stop=True)
                    nc.tensor.matmul(
                        pvB, va[:, NSCH - 1, hB, :], p15[:, 512:1024],
                        start=False, stop=True)
                    un = unpool.tile([65, 2, 512], F32, tag="un", name="un")
                    nc.vector.tensor_copy(un[:, 0, :], pvA)
                    if tail:
                        nc.scalar.activation(un[:, 1, :], pvB, COPY)
                    else:
                        nc.vector.tensor_copy(un[:, 1, :], pvB)
                    return (pair, tq, un)

                prev = None     # quarter awaiting deferred P@V + staging
                pending = None  # quarter awaiting finalize + out_proj
                for pair in range(2):
                    hA, hB = 2 * pair, 2 * pair + 1
                    qc = pair       # Q chunk of this pair
                    kc = 2 + pair   # K chunk
                    for tq in range(4):
                        toff = tq * 512
                        slots = {}
                        for st, work in late_slots.get(pair * 4 + tq, []):
                            slots.setdefault(st, []).append(work)
                        pvA = pvB = None
                        pbuf = {}
                        with nc.named_scope(f"scores{pair}_{tq}"):
                            for i in range(NSCH):
                                with tc.high_priority(offset=48):
                                    sc = scp.tile([128, 1024], F32, tag="sc")
                                    nc.tensor.matmul(
                                        sc[:, 0:512],
                                        qk[0:64, kc, bass.ts(i, 128)],
                                        qk[0:64, qc, bass.ds(toff, 512)],
                                        start=True, stop=True)
                                    nc.tensor.matmul(
                                        sc[:, 512:1024],
                                        qk[64:128, kc, bass.ts(i, 128)],
                                        qk[64:128, qc, bass.ds(toff, 512)],
                                        start=True, stop=True)
                                    p = ppool.tile([128, 1024], F16, tag="p")
                                    nc.scalar.activation(
                                        p, sc, EXP, scale=SCALING)
                                pbuf[i] = p
                                # close the previous quarter across the
                                # boundary, spread over strips 0-1, so its
                                # trailing P@V work and staging copies never
                                # delay this quarter's score pairs.
                                if i == 0 and prev is not None:
                                    close_a(prev)
                                if i == 1 and prev is not None:
                                    pending = close_b(prev)
                                    prev = None
                                # P@V lags the scores by 2 chunks: the PE
                                # queue always holds the next score pair
                                # ahead of P@V, and the new quarter's first
                                # P@V lands after the old banks are staged.
                                if i >= 2:
                                    if i == 2:
                                        pvA = pvp.tile([65, 512], F32,
                                                       tag="pvA", name="pvA")
                                        pvB = pvp.tile([65, 512], F32,
                                                       tag="pvB", name="pvB")
                                    nc.tensor.matmul(
                                        pvA, va[:, i - 2, hA, :],
                                        pbuf[i - 2][:, 0:512],
                                        start=(i == 2), stop=False)
                                    nc.tensor.matmul(
                                        pvB, va[:, i - 2, hB, :],
                                        pbuf[i - 2][:, 512:1024],
                                        start=(i == 2), stop=False)
                                    pbuf.pop(i - 2)
                                if pending is not None and i == 3:
                                    emit_fin_rest(*pending)
                                if pending is not None and i in (10, 11, 12, 13):
                                    emit_oproj_chunk(
                                        pending[0], pending[1],
                                        [2 * (i - 10), 2 * (i - 10) + 1])
                                    if i == 13:
                                        pending = None
                                for work in slots.get(i, ()):
                                    if work[0] == "v":
                                        emit_v_group(work[1], auxp, "aux")
                                    else:
                                        emit_qk_group(*work[1], auxp, "aux",
                                                      half=work[2])
                        prev = (pair, tq, pvA, pvB, hA, hB,
                                pbuf[NSCH - 2], pbuf[NSCH - 1])

                # tail: close the last quarter. The finalize latency
                # chain (denominator gather DMA -> reciprocal -> row
                # scatter DMA -> PE ones-broadcast -> normalize) is bridged
                # with dummy matmuls so the PE HAM clock gate stays open
                # for out_proj.
                def dummies(n):
                    for w in range(n):
                        dmy = scp.tile([128, 1024], F32, tag="sc", name="dmy")
                        nc.tensor.matmul(
                            dmy[:, 0:512], qk[0:64, 0, 0:128],
                            qk[0:64, 0, 0:512], start=True, stop=True)

                close_a(prev)
                pair, tq, un = close_b(prev, tail=True)
                dummies(14)
                with nc.named_scope("fintail"):
                    recin = fpool.tile([128, 8], F32, tag="recin")
                    nc.sync.dma_start(out=recin, in_=un[64:65, :, :])
                    recw = fpool.tile([128, 8], F32, tag="recw")
                    nc.vector.reciprocal(recw, recin)
                    recw16 = fpool.tile([128, 8], F16, tag="recw16")
                    nc.vector.tensor_copy(recw16, recw)
                    dummies(8)
                    recrow = fpool.tile([1, 2, 512], F16, tag="recrow")
                    nc.sync.dma_start(out=recrow, in_=recw16)
                    dummies(12)
                    for h in range(2):
                        bcp = auxp.tile([64, 512], F32, tag="aux",
                                        name=f"bc{h}")
                        nc.tensor.matmul(bcp, onesr, recrow[:, h, :],
                                         start=True, stop=True)
                        nc.vector.tensor_mul(
                            attn[h * 64:h * 64 + 64, pair,
                                 bass.ds(tq * 512, 512)],
                            un[0:64, h, :], bcp)
                emit_oproj_chunk(pair, tq, list(range(NEC)), tail=True)
    _split_excess_waits(nc)
    return nc


_NC_CACHE = None


def _get_nc():
    global _NC_CACHE
    if _NC_CACHE is None:
        _NC_CACHE = _build_nc()
    return _NC_CACHE


def kernel(x, in_proj_weight, in_proj_bias, out_proj_weight, out_proj_bias,
           _run_kwargs=None, _capture=None):
    x = np.asarray(x, dtype=np.float32)
    in_proj_weight = np.asarray(in_proj_weight, dtype=np.float32)
    in_proj_bias = np.asarray(in_proj_bias, dtype=np.float32)
    out_proj_weight = np.asarray(out_proj_weight, dtype=np.float32)
    out_proj_bias = np.asarray(out_proj_bias, dtype=np.float32)

    nc = _get_nc()
    # xT pre-chunked [128, NEC, S]: partition p, chunk c -> feature c*128+p
    xTb = [np.ascontiguousarray(
               x[:, b, :].T.reshape(NEC, 128, S).transpose(1, 0, 2)
           ).astype(np.float16).reshape(128, NEC, S)
           for b in range(B)]

    def shuffle(wT, width):
        # [E, width] -> [128, NEC*width]: row p, col c*width+f = wT[c*128+p, f]
        return np.ascontiguousarray(
            wT.reshape(NEC, 128, width).transpose(1, 0, 2).reshape(128, -1)
        ).astype(np.float16)

    in_maps = []
    for c in range(N_CORES):
        b = c // 4
        h0 = (c % 4) * 4
        rows = slice(h0 * D, h0 * D + 4 * D)
        wq = in_proj_weight[0:E][rows]          # [256, 1024]
        wk = in_proj_weight[E:2 * E][rows]
        wv_ = in_proj_weight[2 * E:3 * E][rows]
        wqkT = np.concatenate([wq, wk], axis=0).T   # [E, 512] fc0|fc1|fc2|fc3
        wqkAS = shuffle(wqkT[:, np.r_[0:128, 256:384]], 256)
        wqkBS = shuffle(wqkT[:, np.r_[128:256, 384:512]], 256)
        wvS = shuffle(wv_.T, 256)
        # woutT [256, E] -> [128, 2*E] (2 chunks of 128)
        woutT = out_proj_weight[:, rows].T
        woutS = np.ascontiguousarray(
            woutT.reshape(2, 128, E).transpose(1, 0, 2).reshape(128, -1)
        ).astype(np.float16)
        # Q bias per 128-feature chunk; K bias is softmax-invariant (dropped),
        # V bias is folded into the output bias host-side.
        bqv = in_proj_bias[0:E][rows]           # [256]
        bias_q = np.ascontiguousarray(bqv.reshape(2, 128).T)
        in_maps.append({
            "xT": xTb[b],
            "wqkAS": wqkAS,
            "wqkBS": wqkBS,
            "wvS": wvS,
            "woutS": woutS,
            "bias_q": bias_q.astype(np.float32),
        })

    res = run_bass_kernel_spmd(nc, in_maps, core_ids=list(range(N_CORES)),
                               **(_run_kwargs or {}))
    if _capture is not None:
        _capture["res"] = res

    out = np.zeros((S, B, E), dtype=np.float32)
    for c in range(N_CORES):
        b = c // 4
        o = res.results[c]["outT"]
        out[:, b, :] += o[0].astype(np.float32).T
        out[:, b, :] += o[1].astype(np.float32).T
    # out_proj bias + the folded V-bias contribution (attn bias bv passes
    # through softmax untouched: P rows sum to 1).
    v_bias = in_proj_bias[2 * E:3 * E]
    out += out_proj_bias + out_proj_weight @ v_bias
    return out


# revision 27
# speedup vs baseline: 1.1930x; 1.1930x over previous
"""MultiHeadAttention forward on 8 TRN2 NeuronCores (batch*head parallel).

Problem: S=2048, B=2, E=1024, H=16 heads, D=64. Each core handles one batch
(b = core//4) and 4 consecutive heads ((core%4)*4 ...), as 2 head-pairs.

v3: float16 data path (numpy-sim metric 4.0e-3 vs the 2e-2 gate; fp16 matmuls
run 1 cycle/row with overlappable fast weight loads, halving DMA bytes and
killing the f32r weight-load serialization). The kernel is paced by the
Scalar engine's exp (~1.12us per [128,1024] strip, 128 strips = 143us floor);
everything else is arranged to keep that engine saturated:

  Warmup: ~7us of dummy matmuls raise the PE HAM clock gate (1.2 -> 2.4 GHz)
    while input DMAs stream; a dummy exp preloads the ACT spline table.
  DMA: weights arrive pre-shuffled so every transfer is fat-descriptor
    contiguous. wqk + x stream on the Activation HWDGE queue (free until the
    first exp), wv/wout/bias on the Sync queue in parallel.
  Phase A (upfront): only what phase B's first quarter needs: K chunks of
    pair 0 (fc2), Q chunk sb0 (fc0), and V. The other 11 projection groups
    run inside phase B's PE slack via the aux PSUM ring, two per quarter.
  Phase B per (pair, tq): per 128-key chunk i: 2 row-packed score matmuls
    [64-contraction, heads at rows 0-63/64-127], one exp over the combined
    [128,1024] PSUM strip (scale 1/8 folded in), 2 P@V accumulations with
    [V|1] stationary (the ones column yields softmax denominators in row 64).
    Score matmuls for chunk i+1 are emitted BEFORE P@V of chunk i so the PE
    queue never head-of-line blocks the next exp strip.
  Finalize: P@V + denominators staged to SBUF right after the last chunk
    (frees PSUM); the rest (denominator shuffle-DMA onto 128 partitions, DVE
    reciprocal, DRAM-bounce broadcast over 64 partitions, 2 multiplies) is
    emitted early in the NEXT quarter, as is out_proj (8 matmuls + copies +
    one batched 2-kick DMA out). K bias is softmax-invariant (dropped); V
    bias is folded into the output bias host-side; Q bias applied on-chip.
  Host: sums the 2x4 fp16 partials per batch in fp32 and adds the folded
    output bias.
"""
import os
import sys

if "/opt/trn_rl_repo" not in sys.path:
    sys.path.insert(0, "/opt/trn_rl_repo")

import numpy as np

import concourse.bass as bass
import concourse.tile as tile
from concourse import mybir
from concourse.bass_utils import run_bass_kernel_spmd

_LDWOPT = os.environ.get("LDWOPT", "0") == "1"
if _LDWOPT:
    import concourse.bass_utils as _bu

    _orig_run_command = _bu.run_command

    def _run_command_ldwopt(argv, **kw):
        argv = ["--enable-ldw-opt=true" if a == "--enable-ldw-opt=false" else a
                for a in argv]
        return _orig_run_command(argv, **kw)

    _bu.run_command = _run_command_ldwopt

S = 2048
B = 2
E = 1024
H = 16
D = 64
N_CORES = 8
F32 = mybir.dt.float32
F16 = mybir.dt.float16
EXP = mybir.ActivationFunctionType.Exp
COPY = mybir.ActivationFunctionType.Copy
SCALING = float(D) ** -0.5

NSCH = S // 128   # 16 s-chunks
NSB = S // 512    # 4 s-blocks
NEC = E // 128    # 8 e-chunks


def _split_excess_waits(nc, limit=1):
    """This walrus build accepts at most 2 sync-wait commands per instruction;
    hoist excess waits onto preceding same-engine NOPs (queue order preserves
    semantics)."""
    ctr = 0
    for f in nc.m.functions:
        for blk in f.blocks:
            insts = blk.instructions
            if not any(
                i.sync_info and i.sync_info.on_wait and len(i.sync_info.on_wait) > limit
                for i in insts
            ):
                continue
            out = []
            for inst in insts:
                si = inst.sync_info
                if si and si.on_wait and len(si.on_wait) > limit:
                    waits = list(si.on_wait)
                    excess, keep = waits[:-limit], waits[-limit:]
                    for i in range(0, len(excess), limit):
                        ctr += 1
                        nop = mybir.InstNoOp(name=f"waitsplit-nop-{ctr}")
                        nop.engine = inst.engine
                        nop.sync_info = mybir.SyncInfo(
                            on_wait=excess[i : i + limit], on_update=[]
                        )
                        nc.register_instruction(nop, overwrite=True)
                        out.append(nop)
                    si.on_wait = keep
                out.append(inst)
            blk.instructions.clear()
            blk.instructions.extend(out)
    return nc


def _build_nc():
    nc = bass.Bass()
    xT = nc.dram_tensor("xT", [128, NEC, S], F16, kind="ExternalInput")
    wqkAS = nc.dram_tensor("wqkAS", [128, NEC * 256], F16, kind="ExternalInput")
    wqkBS = nc.dram_tensor("wqkBS", [128, NEC * 256], F16, kind="ExternalInput")
    wvS = nc.dram_tensor("wvS", [128, NEC * 256], F16, kind="ExternalInput")
    woutS = nc.dram_tensor("woutS", [128, 2 * E], F16, kind="ExternalInput")
    bias_q = nc.dram_tensor("bias_q", [128, 2], F32, kind="ExternalInput")
    outT = nc.dram_tensor("outT", [2, E, S], F16, kind="ExternalOutput")
    # DRAM scratch for the reciprocal rows (SBUF->SBUF broadcast DMA is not
    # supported; DRAM->SBUF broadcast is): one row per (pair, tq).
    recd = nc.dram_tensor("recd", [8, 2, 512], F32, kind="Internal")

    with tile.TileContext(nc) as tc, \
         tc.tile_pool(name="xpool", bufs=1) as xpool, \
         tc.tile_pool(name="wpool", bufs=1) as wpool:
        # ---- input DMA kicks. The hardware DMA rings share ~0.38 MB/us of
        # aggregate bandwidth and the Activation ring cannot start before
        # ~11.5us, so the critical stream (wqk then x, in the exact ec order
        # the in-order QK accumulation consumes) rides the Sync queue; the
        # small late-use weights ride the Activation queue.
        xt = xpool.tile([128, NEC, S], F16)
        wqkA = wpool.tile([128, NEC, 256], F16)   # fc0 | fc2
        wqkB = wpool.tile([128, NEC, 256], F16)   # fc1 | fc3
        wv = wpool.tile([128, NEC, 256], F16)
        bq = wpool.tile([128, 2], F32)
        wout = wpool.tile([128, 2, E], F16)
        nc.sync.dma_start(
            out=wqkA, in_=wqkAS.rearrange("p (c f) -> p c f", c=NEC))
        for ec in range(NEC):
            nc.sync.dma_start(out=xt[:, ec, :], in_=xT[:, ec, :])
        nc.sync.dma_start(
            out=wqkB, in_=wqkBS.rearrange("p (c f) -> p c f", c=NEC))
        nc.scalar.dma_start(out=bq, in_=bias_q[:, :])
        nc.scalar.dma_start(
            out=wv, in_=wvS.rearrange("p (c f) -> p c f", c=NEC))
        nc.scalar.dma_start(
            out=wout, in_=woutS.rearrange("p (c f) -> p c f", c=2))

        def wsel(fc, ec):
            t = wqkA if fc in (0, 2) else wqkB
            return t[:, ec, bass.ds((fc // 2) * 128, 128)]

        # ---- warmup: dummy matmuls (~7us at the cold clock) raise the HAM
        # clock gate to 2.4 GHz while the input DMAs stream; a dummy exp
        # preloads the ACT spline table. No DMA dependencies.
        with tc.tile_pool(name="wupool", bufs=1) as wup, \
             tc.tile_pool(name="wupsum", bufs=2, space="PSUM") as wups:
            wu = wup.tile([128, 512], F16)
            with nc.named_scope(f"warmup_ldwopt{int(_LDWOPT)}"):
                nc.vector.memset(wu, 1.0)
            wusink = wup.tile([128, 512], F32)
            for w in range(16):
                ps = wups.tile([128, 512], F32, tag="wu")
                nc.tensor.matmul(ps, wu[:, 0:128], wu, start=True, stop=True)
                if w == 15:
                    nc.scalar.activation(wusink, ps, EXP, scale=0.001)

        with tc.tile_pool(name="qkpool", bufs=1) as qkpool, \
             tc.tile_pool(name="vapool", bufs=1) as vapool, \
             tc.tile_pool(name="attnpool", bufs=1) as attnpool, \
             tc.tile_pool(name="ppool", bufs=4) as ppool:
            ones64 = wpool.tile([128, 64], F32)
            nc.vector.memset(ones64, 1.0)
            onesr = wpool.tile([1, 64], F16)
            nc.vector.memset(onesr, 1.0)

            # persistent activations
            qk = qkpool.tile([128, 4, S], F16)         # Q^T (chunks 0-1), K^T (2-3)
            va = vapool.tile([128, NSCH, 4, 65], F16)  # V natural + ones col
            attn = attnpool.tile([128, 2, S], F16)     # attn^T normalized

            nc.vector.tensor_copy(
                va[:, :, :, 64:65],
                ones64.rearrange("p (c h) -> p c h", h=4).unsqueeze(3))

            qkg_state = {}

            def emit_qk_group(fc, sb, pool, tag, half=None):
                """QKV projection group; half=0/1 splits the 8-MM
                accumulation into two 4-MM batches (same psum tile) so a
                group fits a strip's PE slack without bursting."""
                if half in (None, 0):
                    qkg_state[(fc, sb)] = pool.tile(
                        [128, 512], F32, tag=tag, name=f"qk{fc}_{sb}")
                ps = qkg_state[(fc, sb)]
                ecs = range(NEC) if half is None else (
                    range(4) if half == 0 else range(4, NEC))
                for ec in ecs:
                    nc.tensor.matmul(
                        ps,
                        wsel(fc, ec),
                        xt[:, ec, bass.ts(sb, 512)],
                        start=(ec == 0), stop=(ec == NEC - 1))
                if half == 0:
                    return
                if fc < 2:  # Q chunk: add bias
                    nc.vector.tensor_scalar(
                        out=qk[:, fc, bass.ts(sb, 512)], in0=ps,
                        scalar1=bq[:, fc:fc + 1], scalar2=None,
                        op0=mybir.AluOpType.add)
                else:       # K chunk: bias dropped (softmax-invariant)
                    nc.vector.tensor_copy(qk[:, fc, bass.ts(sb, 512)], ps)

            def emit_v_group(i, pool, tag):
                ps = pool.tile([128, 512], F32, tag=tag, name=f"v{i}")
                for ec in range(NEC):
                    nc.tensor.matmul(
                        ps[:, 0:256],
                        xt[:, ec, bass.ts(i, 128)],
                        wv[:, ec, :],
                        start=(ec == 0), stop=(ec == NEC - 1))
                nc.vector.tensor_copy(
                    va[:, i, :, 0:64],
                    ps[:, 0:256].rearrange("p (h d) -> p h d", h=4))

            # ---- phase A (upfront): only what phase B quarter 0's first
            # strip needs; V and the other 11 projection groups are spread
            # through phase B's strip slots via the aux ring.
            # One PSUM slot per group and ec-wave emission order: each
            # arriving x chunk feeds every group immediately (the in-order
            # PE queue would otherwise serialize whole groups behind the
            # slowest chunk).
            with nc.named_scope("phaseA"), \
                 tc.tile_pool(name="apsum", bufs=1, space="PSUM") as apsum:
                agroups = [(0, 0), (2, 0), (2, 1), (2, 2)]
                aps = {
                    (fc, sb): apsum.tile([128, 512], F32, tag=f"a{fc}_{sb}",
                                         name=f"aps{fc}_{sb}")
                    for fc, sb in agroups
                }
                for ec in range(NEC):
                    for fc, sb in agroups:
                        nc.tensor.matmul(
                            aps[(fc, sb)],
                            wsel(fc, ec),
                            xt[:, ec, bass.ts(sb, 512)],
                            start=(ec == 0), stop=(ec == NEC - 1))
                for fc, sb in agroups:
                    if fc < 2:
                        nc.vector.tensor_scalar(
                            out=qk[:, fc, bass.ts(sb, 512)],
                            in0=aps[(fc, sb)],
                            scalar1=bq[:, fc:fc + 1], scalar2=None,
                            op0=mybir.AluOpType.add)
                    else:
                        nc.vector.tensor_copy(
                            qk[:, fc, bass.ts(sb, 512)], aps[(fc, sb)])

            # late work items: ("v", i) or ("qk", (fc, sb), half).
            # The in-order PE queue guarantees each lands before its
            # consumer; the strip placement only controls stalls.
            def qk_halves(fc, sb, s0, s1):
                return [(s0, ("qk", (fc, sb), 0)), (s1, ("qk", (fc, sb), 1))]

            late_slots = {
                0: [(i, ("v", i)) for i in range(NSCH)]
                   + qk_halves(2, 3, 5, 6)
                   + qk_halves(0, 1, 12, 13),
                1: qk_halves(0, 2, 7, 8) + qk_halves(0, 3, 11, 12),
                2: qk_halves(3, 0, 7, 8) + qk_halves(3, 1, 11, 12),
                3: qk_halves(3, 2, 6, 7) + qk_halves(3, 3, 8, 9)
                   + qk_halves(1, 0, 12, 13),
                4: qk_halves(1, 1, 7, 8) + qk_halves(1, 2, 11, 12),
                5: qk_halves(1, 3, 7, 8),
            }

            # ---- phase B + deferred finalize/out_proj ----
            with tc.tile_pool(name="unpool", bufs=3) as unpool, \
                 tc.tile_pool(name="fpool", bufs=2) as fpool, \
                 tc.tile_pool(name="opool", bufs=2) as opool, \
                 tc.tile_pool(name="scpsum", bufs=2, space="PSUM") as scp, \
                 tc.tile_pool(name="pvpsum", bufs=1, space="PSUM") as pvp, \
                 tc.tile_pool(name="auxpsum", bufs=2, space="PSUM") as auxp:

                def emit_fin_rest(pair, tq, un):
                    """Denominator -> reciprocal -> broadcast -> normalize."""
                    with nc.named_scope(f"fin{pair}_{tq}"):
                        recin = fpool.tile([128, 8], F32, tag="recin")
                        nc.sync.dma_start(out=recin, in_=un[64:65, :, :])
                        recw = fpool.tile([128, 8], F32, tag="recw")
                        nc.vector.reciprocal(recw, recin)
                        ridx = pair * 4 + tq
                        nc.sync.dma_start(out=recd[ridx], in_=recw)
                        recb = fpool.tile([64, 2, 512], F32, tag="recb")
                        nc.sync.dma_start(
                            out=recb,
                            in_=recd[ridx].unsqueeze(0)
                            .to_broadcast([64, 2, 512]))
                        for h in range(2):
                            prt = h * 64
                            nc.vector.tensor_mul(
                                attn[prt:prt + 64, pair,
                                     bass.ds(tq * 512, 512)],
                                un[0:64, h, :],
                                recb[:, h, :])

                oproj_state = {}

                def emit_oproj_chunk(pair, tq, fcs, tail=False):
                    toff = tq * 512
                    with nc.named_scope(f"oproj{pair}_{tq}"):
                        if fcs[0] == 0:
                            oproj_state["ocp"] = opool.tile(
                                [128, NEC, 512], F16, tag="ocp", name="ocp")
                        ocp = oproj_state["ocp"]
                        for fc in fcs:
                            ps = auxp.tile([128, 512], F32, tag="aux",
                                           name=f"op{pair}_{tq}_{fc}")
                            nc.tensor.matmul(
                                ps,
                                wout[:, pair, bass.ts(fc, 128)],
                                attn[:, pair, bass.ds(toff, 512)],
                                start=True, stop=True)
                            if tail and fc % 2 == 1:
                                nc.scalar.activation(ocp[:, fc, :], ps, COPY)
                            else:
                                nc.vector.tensor_copy(ocp[:, fc, :], ps)
                        if fcs[-1] == NEC - 1:
                            outv = outT[pair].rearrange(
                                "(c p) s -> p c s", p=128)
                            nk = 4 if tail else 2
                            step = NEC // nk
                            for k in range(nk):
                                nc.sync.dma_start(
                                    out=outv[:, bass.ds(k * step, step),
                                             bass.ds(toff, 512)],
                                    in_=ocp[:, bass.ds(k * step, step), :])

                def close_a(prev):
                    """Deferred P@V of chunk 14 (emitted after the next
                    quarter's first score pair)."""
                    pair, tq, pvA, pvB, hA, hB, p14, p15 = prev
                    nc.tensor.matmul(
                        pvA, va[:, NSCH - 2, hA, :], p14[:, 0:512],
                        start=False, stop=False)
                    nc.tensor.matmul(
                        pvB, va[:, NSCH - 2, hB, :], p14[:, 512:1024],
                        start=False, stop=False)

                def close_b(prev, tail=False):
                    """Deferred P@V of chunk 15 + SBUF staging; frees the
                    P@V PSUM banks for the next quarter's accumulation."""
                    pair, tq, pvA, pvB, hA, hB, p14, p15 = prev
                    nc.tensor.matmul(
                        pvA, va[:, NSCH - 1, hA, :], p15[:, 0:512],
                        start=False, stop=True)
                    nc.tensor.matmul(
                        pvB, va[:, NSCH - 1, hB, :], p15[:, 512:1024],
                        start=False, stop=True)
                    un = unpool.tile([65, 2, 512], F32, tag="un", name="un")
                    nc.vector.tensor_copy(un[:, 0, :], pvA)
                    if tail:
                        nc.scalar.activation(un[:, 1, :], pvB, COPY)
                    else:
                        nc.vector.tensor_copy(un[:, 1, :], pvB)
                    return (pair, tq, un)

                prev = None     # quarter awaiting deferred P@V + staging
                pending = None  # quarter awaiting finalize + out_proj
                for pair in range(2):
                    hA, hB = 2 * pair, 2 * pair + 1
                    qc = pair       # Q chunk of this pair
                    kc = 2 + pair   # K chunk
                    for tq in range(4):
                        toff = tq * 512
                        slots = {}
                        for st, work in late_slots.get(pair * 4 + tq, []):
                            slots.setdefault(st, []).append(work)
                        pvA = pvB = None
                        pbuf = {}
                        with nc.named_scope(f"scores{pair}_{tq}"):
                            for i in range(NSCH):
                                with tc.high_priority(offset=48):
                                    sc = scp.tile([128, 1024], F32, tag="sc")
                                    nc.tensor.matmul(
                                        sc[:, 0:512],
                                        qk[0:64, kc, bass.ts(i, 128)],
                                        qk[0:64, qc, bass.ds(toff, 512)],
                                        start=True, stop=True)
                                    nc.tensor.matmul(
                                        sc[:, 512:1024],
                                        qk[64:128, kc, bass.ts(i, 128)],
                                        qk[64:128, qc, bass.ds(toff, 512)],
                                        start=True, stop=True)
                                    p = ppool.tile([128, 1024], F16, tag="p")
                                    nc.scalar.activation(
                                        p, sc, EXP, scale=SCALING)
                                pbuf[i] = p
                                # close the previous quarter across the
                                # boundary, spread over strips 0-1, so its
                                # trailing P@V work and staging copies never
                                # delay this quarter's score pairs.
                                if i == 0 and prev is not None:
                                    close_a(prev)
                                if i == 1 and prev is not None:
                                    pending = close_b(prev)
                                    prev = None
                                # P@V lags the scores by 2 chunks: the PE
                                # queue always holds the next score pair
                                # ahead of P@V, and the new quarter's first
                                # P@V lands after the old banks are staged.
                                if i >= 2:
                                    if i == 2:
                                        pvA = pvp.tile([65, 512], F32,
                                                       tag="pvA", name="pvA")
                                        pvB = pvp.tile([65, 512], F32,
                                                       tag="pvB", name="pvB")
                                    nc.tensor.matmul(
                                        pvA, va[:, i - 2, hA, :],
                                        pbuf[i - 2][:, 0:512],
                                        start=(i == 2), stop=False)
                                    nc.tensor.matmul(
                                        pvB, va[:, i - 2, hB, :],
                                        pbuf[i - 2][:, 512:1024],
                                        start=(i == 2), stop=False)
                                    pbuf.pop(i - 2)
                                if pending is not None and i == 3:
                                    emit_fin_rest(*pending)
                                if pending is not None and i in (10, 11, 12, 13):
                                    emit_oproj_chunk(
                                        pending[0], pending[1],
                                        [2 * (i - 10), 2 * (i - 10) + 1])
                                    if i == 13:
                                        pending = None
                                for work in slots.get(i, ()):
                                    if work[0] == "v":
                                        emit_v_group(work[1], auxp, "aux")
                                    else:
                                        emit_qk_group(*work[1], auxp, "aux",
                                                      half=work[2])
                        prev = (pair, tq, pvA, pvB, hA, hB,
                                pbuf[NSCH - 2], pbuf[NSCH - 1])

                # tail: close the last quarter. The finalize latency
                # chain (denominator gather DMA -> reciprocal -> row
                # scatter DMA -> PE ones-broadcast -> normalize) is bridged
                # with dummy matmuls so the PE HAM clock gate stays open
                # for out_proj.
                def dummies(n):
                    for w in range(n):
                        dmy = scp.tile([128, 1024], F32, tag="sc", name="dmy")
                        nc.tensor.matmul(
                            dmy[:, 0:512], qk[0:64, 0, 0:128],
                            qk[0:64, 0, 0:512], start=True, stop=True)

                close_a(prev)
                pair, tq, un = close_b(prev, tail=True)
                dummies(14)
                with nc.named_scope("fintail"):
                    recin = fpool.tile([128, 8], F32, tag="recin")
                    nc.sync.dma_start(out=recin, in_=un[64:65, :, :])
                    recw = fpool.tile([128, 8], F32, tag="recw")
                    nc.vector.reciprocal(recw, recin)
                    recw16 = fpool.tile([128, 8], F16, tag="recw16")
                    nc.vector.tensor_copy(recw16, recw)
                    dummies(8)
                    recrow = fpool.tile([1, 2, 512], F16, tag="recrow")
                    nc.sync.dma_start(out=recrow, in_=recw16)
                    for h in range(2):
                        bcp = auxp.tile([64, 512], F32, tag="aux",
                                        name=f"bc{h}")
                        nc.tensor.matmul(bcp, onesr, recrow[:, h, :],
                                         start=True, stop=True)
                        nc.vector.tensor_mul(
                            attn[h * 64:h * 64 + 64, pair,
                                 bass.ds(tq * 512, 512)],
                            un[0:64, h, :], bcp)
                emit_oproj_chunk(pair, tq, list(range(NEC)), tail=True)
    _split_excess_waits(nc)
    return nc


_NC_CACHE = None


def _get_nc():
    global _NC_CACHE
    if _NC_CACHE is None:
        _NC_CACHE = _build_nc()
    return _NC_CACHE


def kernel(x, in_proj_weight, in_proj_bias, out_proj_weight, out_proj_bias,
           _run_kwargs=None, _capture=None):
    x = np.asarray(x, dtype=np.float32)
    in_proj_weight = np.asarray(in_proj_weight, dtype=np.float32)
    in_proj_bias = np.asarray(in_proj_bias, dtype=np.float32)
    out_proj_weight = np.asarray(out_proj_weight, dtype=np.float32)
    out_proj_bias = np.asarray(out_proj_bias, dtype=np.float32)

    nc = _get_nc()
    # xT pre-chunked [128, NEC, S]: partition p, chunk c -> feature c*128+p
    xTb = [np.ascontiguousarray(
               x[:, b, :].T.reshape(NEC, 128, S).transpose(1, 0, 2)
           ).astype(np.float16).reshape(128, NEC, S)
           for b in range(B)]

    def shuffle(wT, width):
        # [E, width] -> [128, NEC*width]: row p, col c*width+f = wT[c*128+p, f]
        return np.ascontiguousarray(
            wT.reshape(NEC, 128, width).transpose(1, 0, 2).reshape(128, -1)
        ).astype(np.float16)

    in_maps = []
    for c in range(N_CORES):
        b = c // 4
        h0 = (c % 4) * 4
        rows = slice(h0 * D, h0 * D + 4 * D)
        wq = in_proj_weight[0:E][rows]          # [256, 1024]
        wk = in_proj_weight[E:2 * E][rows]
        wv_ = in_proj_weight[2 * E:3 * E][rows]
        wqkT = np.concatenate([wq, wk], axis=0).T   # [E, 512] fc0|fc1|fc2|fc3
        wqkAS = shuffle(wqkT[:, np.r_[0:128, 256:384]], 256)
        wqkBS = shuffle(wqkT[:, np.r_[128:256, 384:512]], 256)
        wvS = shuffle(wv_.T, 256)
        # woutT [256, E] -> [128, 2*E] (2 chunks of 128)
        woutT = out_proj_weight[:, rows].T
        woutS = np.ascontiguousarray(
            woutT.reshape(2, 128, E).transpose(1, 0, 2).reshape(128, -1)
        ).astype(np.float16)
        # Q bias per 128-feature chunk; K bias is softmax-invariant (dropped),
        # V bias is folded into the output bias host-side.
        bqv = in_proj_bias[0:E][rows]           # [256]
        bias_q = np.ascontiguousarray(bqv.reshape(2, 128).T)
        in_maps.append({
            "xT": xTb[b],
            "wqkAS": wqkAS,
            "wqkBS": wqkBS,
            "wvS": wvS,
            "woutS": woutS,
            "bias_q": bias_q.astype(np.float32),
        })

    res = run_bass_kernel_spmd(nc, in_maps, core_ids=list(range(N_CORES)),
                               **(_run_kwargs or {}))
    if _capture is not None:
        _capture["res"] = res

    out = np.zeros((S, B, E), dtype=np.float32)
    for c in range(N_CORES):
        b = c // 4
        o = res.results[c]["outT"]
        out[:, b, :] += o[0].astype(np.float32).T
        out[:, b, :] += o[1].astype(np.float32).T
    # out_proj bias + the folded V-bias contribution (attn bias bv passes
    # through softmax untouched: P rows sum to 1).
    v_bias = in_proj_bias[2 * E:3 * E]
    out += out_proj_bias + out_proj_weight @ v_bias
    return out
